# revision 57
# baseline (speedup 1.0000x reference)
"""Trainium2 Bass kernel for MultiHeadDilatedAttention (v4).

Full inputs in, full output out. 8 cores = (batch b 0..3) x (s-half). Each
core handles b = c//2, 64 of 128 s values; output rows t = s*64 + o form a
contiguous 4096-row chunk of y[b].

Design:
  - x host-cast to bf16, DMA'd in 4 n-chunks interleaved with weights in
    need-order so the PE starts at ~8us instead of ~40us.
  - QKV stored s-major (col = s*L + l). Heads 0/1 projected per chunk;
    heads 2/3 (whose 512-col tiles span chunks) projected fat at the end.
  - V^T -> V-natural via dense slot packing (slot stride = L): 60 full
    [128,128] transposes.
  - Attention packs g=128/L s-values per matmul: ONE KQ matmul (N=128) and
    ONE att matmul (N=128) per group. Off-diagonal (cross-s) garbage blocks
    are killed by a binary mask multiply AFTER exp (mask=0 there), so row
    sums and the att contraction stay correct. 4 groups share one PSUM bank
    ("supergroup"); softmax vector ops run at [128,512] granularity. The
    supergroup loop is software-pipelined (KQ of sg+1 emitted before att of
    sg) so the in-order PE has independent work during the softmax chain.
  - Output projection in y^T orientation (stationary = W_out chunk, moving
    = atT gather) grouped by dilation class: rows t=s*64+o only get matmuls
    from heads with dil | o (120 N=512 matmuls vs 256). Stores go straight
    from PSUM to HBM (fp32); the bias is added on the host. E tiles are
    interleaved into the D pipeline as their atT s-ranges complete.
"""

import os
from contextlib import ExitStack

import numpy as np
import ml_dtypes

import concourse.bass as bass
import concourse.mybir as mybir
import concourse.tile as tile
from concourse import bacc
from concourse.masks import make_identity
from concourse.bass_utils import run_bass_kernel_spmd

F32 = mybir.dt.float32
BF16 = mybir.dt.bfloat16
AX = mybir.AxisListType

B, T, E = 4, 8192, 1024
SEG = 128          # positions per segment (s)
NB = T // SEG      # 64 segments (n)
NS = 64            # s values per core
ROWS = NB * NS     # 4096 rows per core
DK = 128
H = 4
DILS = [1, 2, 4, 8]
LS = [NB // d for d in DILS]       # [64, 32, 16, 8]
G = [128 // l for l in LS]         # s per group: [2, 4, 8, 16]
NG = [NS // g for g in G]          # groups/head: [32, 16, 8, 4]
NSG = [n // 4 for n in NG]         # supergroups: [8, 4, 2, 1]
NORM = float(1.0 / np.sqrt(DK))
NECHUNK = E // 128                 # 8
NCH = 4                            # x n-chunks
CHN = NB // NCH                    # 16 n per chunk

# output classes: offsets o in [0,64) grouped by which heads hit them
CLASSES = [
    (list(range(1, 64, 2)), [0]),           # odd
    (list(range(2, 64, 4)), [0, 1]),        # 2 mod 4
    (list(range(4, 64, 8)), [0, 1, 2]),     # 4 mod 8
    (list(range(0, 64, 8)), [0, 1, 2, 3]),  # 0 mod 8
]
# per-class: (n_tiles, s_per_tile, o_start, o_step, n_o)
CLS_TILE = [(4, 16, 1, 2, 32), (2, 32, 2, 4, 16), (1, 64, 4, 8, 8),
            (1, 64, 0, 8, 8)]


def _col_perm():
    """Packed yT column j -> row t_local = s*64 + o, in emission order."""
    perm = []
    for ci, (ntl, spt, o0, ostep, no) in enumerate(CLS_TILE):
        for tl in range(ntl):
            for s in range(tl * spt, (tl + 1) * spt):
                for o in range(o0, 64, ostep):
                    perm.append(s * 64 + o)
    assert len(perm) == 4096
    return np.array(perm, np.int64)


COL_PERM = _col_perm()


def build_program() -> bass.Bass:
    nc = bacc.Bacc("TRN2", target_bir_lowering=False, debug=False)
    # x chunks: [chunk, ec, 128, CHN*64] bf16
    xs = nc.dram_tensor("xs", [NCH, NECHUNK, 128, CHN * NS], BF16,
                        kind="ExternalInput").ap()
    wqkv = nc.dram_tensor("wqkv", [128, 12 * NECHUNK * 128], BF16,
                          kind="ExternalInput").ap()
    wout = nc.dram_tensor("wout", [128, H * E], BF16,
                          kind="ExternalInput").ap()
    bmd = nc.dram_tensor("bm", [128, H * 128], BF16, kind="ExternalInput").ap()
    yT = nc.dram_tensor("yT", [NECHUNK, 128, ROWS], BF16,
                        kind="ExternalOutput").ap()
    _build(nc, xs, wqkv, wout, bmd, yT)
    nc.finalize()
    return nc


def _build(nc, xs, wqkv, wout, bmd, yT):
    with ExitStack() as ctx:
        tc = ctx.enter_context(tile.TileContext(nc))

        persist = ctx.enter_context(tc.tile_pool(name="persist", bufs=1))
        ident = persist.tile([128, 128], BF16, tag="ident")
        make_identity(nc, ident)
        w_sb = persist.tile([128, 12 * NECHUNK * 128], BF16, tag="w_sb")
        wout_sb = persist.tile([128, H * E], BF16, tag="wout_sb")
        bm_sb = persist.tile([128, H * 128], BF16, tag="bm_sb")

        # persistent per-head Q^T/K^T/V^T, s-major: col = s*L + l
        qkvpool = ctx.enter_context(tc.tile_pool(name="qkv", bufs=1))
        qkv_sb = {}
        for h in range(H):
            for p in range(3):
                qkv_sb[(h, p)] = qkvpool.tile(
                    [128, LS[h] * NS], BF16, tag=f"qkv{h}{p}",
                    name=f"qkv{h}{p}")

        xt_pool = ctx.enter_context(tc.tile_pool(name="xt", bufs=1))
        xt = [xt_pool.tile([128, ROWS], BF16, tag=f"xt{ec}",
                           name=f"xt{ec}") for ec in range(NECHUNK)]

        def cast_copy(out_ap, in_ap, eng="scalar"):
            if eng == "scalar":
                nc.scalar.copy(out=out_ap, in_=in_ap)
            else:
                nc.vector.tensor_copy(out=out_ap, in_=in_ap)

        # ---------------- phase B: QKV projection ---------------------------
        # w_sb col ((h*3+p)*8+ec)*128 holds W^T[e-chunk ec] for head h, proj p
        def wsl(h, p, ec):
            i = ((h * 3 + p) * NECHUNK + ec) * 128
            return w_sb[:, i:i + 128]

        with ExitStack() as pctx:
            # DMA in need-order on one queue: first half of w(h0,k), x c0,
            # rest of w(h0), w(h1), x c1, w(h2)+w(h3), x c2/c3, late tensors.
            PB = NECHUNK * 128       # wqkv cols per (head, proj)
            def dma_w(h, p):
                i = (h * 3 + p) * PB
                nc.sync.dma_start(out=w_sb[:, i:i + PB],
                                  in_=wqkv[:, i:i + PB])
            def dma_x(c):
                for ec in range(NECHUNK):
                    nc.sync.dma_start(
                        out=xt[ec][:, c * CHN * NS:(c + 1) * CHN * NS],
                        in_=xs[c, ec, :, :])
            i01 = 1 * PB
            nc.sync.dma_start(out=w_sb[:, i01:i01 + PB // 2],
                              in_=wqkv[:, i01:i01 + PB // 2])
            dma_x(0)
            nc.sync.dma_start(out=w_sb[:, i01 + PB // 2:i01 + PB],
                              in_=wqkv[:, i01 + PB // 2:i01 + PB])
            dma_w(0, 0)
            dma_w(0, 2)
            for p in range(3):
                dma_w(1, p)
            dma_x(1)
            for h in (2, 3):
                for p in range(3):
                    dma_w(h, p)
            dma_x(2)
            dma_x(3)
            nc.sync.dma_start(out=wout_sb, in_=wout)
            nc.sync.dma_start(out=bm_sb, in_=bmd)

            qk_ps = pctx.enter_context(
                tc.tile_pool(name="qk_ps", bufs=4, space="PSUM"))

            copy_flip = [0]
            def proj_tiles(specs):
                """Sequential 8-MM accumulation chains, one per tile."""
                for (h, p, l0, nl, rhs_fn) in specs:
                    L = LS[h]
                    ps = qk_ps.tile([128, nl * NS], F32, tag="qk", name="qk")
                    for ec in range(NECHUNK):
                        nc.tensor.matmul(ps, wsl(h, p, ec), rhs_fn(ec),
                                         start=(ec == 0),
                                         stop=(ec == NECHUNK - 1))
                    # psum cols (l, s) -> sbuf col s*L + l
                    out_ap = qkv_sb[(h, p)].rearrange(
                        "p (s l) -> p l s", l=L)[:, l0:l0 + nl, :]
                    in_ap = ps.rearrange("p (l s) -> p l s", s=NS)
                    cast_copy(out_ap, in_ap,
                              "scalar" if copy_flip[0] % 2 else "vector")
                    copy_flip[0] += 1

            for c in range(NCH):
                xr = [xt[ec].rearrange("p (n s) -> p n s", s=NS)
                      for ec in range(NECHUNK)]
                n0 = c * CHN
                for p in (1, 0, 2):            # k, q, v
                    proj_tiles([
                        (0, p, n0, 8,
                         lambda ec, n0=n0: xr[ec][:, n0:n0 + 8, :]),
                        (0, p, n0 + 8, 8,
                         lambda ec, n0=n0: xr[ec][:, n0 + 8:n0 + 16, :]),
                        (1, p, n0 // 2, 8,
                         lambda ec, n0=n0: xr[ec][:, n0:n0 + CHN:2, :]),
                    ])
            # tail: heads 3 then 2, fat N=512 tiles spanning all chunks
            xr = [xt[ec].rearrange("p (n s) -> p n s", s=NS)
                  for ec in range(NECHUNK)]
            proj_tiles([(3, p, 0, 8, lambda ec: xr[ec][:, 0:NB:8, :])
                        for p in (1, 0, 2)])
            for tl in range(2):
                proj_tiles([
                    (2, p, tl * 8, 8,
                     lambda ec, tl=tl: xr[ec][:, tl * 32:(tl + 1) * 32:4, :])
                    for p in (1, 0, 2)])

        # ---------------- phases C+D+E interleaved --------------------------
        # vnat[h]: [128, NG*128] bf16, group gi cols [gi*128,+128), partition
        # slot = k*L + l for s = gi*g + k  (dense, no pad)
        vnpool = ctx.enter_context(tc.tile_pool(name="vnat", bufs=1))
        vnat = [vnpool.tile([128, NG[h] * 128], BF16, tag=f"vnat{h}",
                            name=f"vnat{h}") for h in range(H)]
        # atT[h]: [128, NS*L] bf16, col = s*L + l (packed; att^T rows=v)
        atpool = ctx.enter_context(tc.tile_pool(name="atT", bufs=1))
        atT = [atpool.tile([128, NS * LS[h]], BF16, tag=f"atT{h}",
                           name=f"atT{h}") for h in range(H)]

        tps = ctx.enter_context(tc.tile_pool(name="tps", bufs=1, space="PSUM"))
        kq_ps = ctx.enter_context(
            tc.tile_pool(name="kq_ps", bufs=3, space="PSUM"))
        at_ps = ctx.enter_context(
            tc.tile_pool(name="at_ps", bufs=2, space="PSUM"))
        y_ps = ctx.enter_context(
            tc.tile_pool(name="y_ps", bufs=2, space="PSUM"))
        sm_pool = ctx.enter_context(tc.tile_pool(name="sm", bufs=3))
        small = ctx.enter_context(tc.tile_pool(name="small", bufs=4))
        yo_pool = ctx.enter_context(tc.tile_pool(name="y_sb", bufs=4))

        def phase_c(h):
            """V^T (s-major) -> V natural groups via [128,128] transposes."""
            vt = qkv_sb[(h, 2)]
            for q4 in range(NG[h] // 4):
                pt = tps.tile([128, 512], BF16, tag="pt")
                for q in range(4):
                    gi = q4 * 4 + q
                    nc.tensor.transpose(pt[:, q * 128:(q + 1) * 128],
                                        vt[:, gi * 128:gi * 128 + 128],
                                        ident)
                cast_copy(vnat[h][:, q4 * 512:(q4 + 1) * 512], pt,
                          "vector" if q4 % 2 else "scalar")

        ecopy_flip = [0]
        def phase_e_tile(ci, tl, col0):
            """One output-class tile: all 8 e-chunks, psum [128,512] each.
            Plain PSUM->SBUF bf16 casts (bias is added on the host)."""
            ntl, spt, o0, ostep, no = CLS_TILE[ci]
            heads = CLASSES[ci][1]
            s0 = tl * spt
            for ech in range(NECHUNK):
                ps = y_ps.tile([128, 512], F32, tag="y")
                for ih, h in enumerate(heads):
                    L, dil = LS[h], DILS[h]
                    rhs = atT[h].rearrange("p (s j) -> p s j", j=L)[
                        :, s0:s0 + spt, (o0 // dil)::(max(ostep // dil, 1))]
                    lhsT = wout_sb[:, h * E + ech * 128:
                                   h * E + ech * 128 + 128]
                    nc.tensor.matmul(ps, lhsT, rhs, start=(ih == 0),
                                     stop=(ih == len(heads) - 1))
                y_sb = yo_pool.tile([128, 512], BF16, tag="ysb", name="ysb")
                cast_copy(y_sb, ps,
                          "scalar" if ecopy_flip[0] % 4 == 3 else "vector")
                ecopy_flip[0] += 1
                nc.sync.dma_start(out=yT[ech, :, col0:col0 + 512], in_=y_sb)

        def phase_d_kq(h, sg):
            """KQ matmuls + softmax chain for one supergroup; returns smkq."""
            kt, qt = qkv_sb[(h, 1)], qkv_sb[(h, 0)]   # s-major: blocks contig
            bmh = bm_sb[:, h * 128:(h + 1) * 128]
            bm_rep = bass.AP(tensor=bmh.tensor, offset=bmh.offset,
                             ap=[bmh.ap[0], [0, 4], bmh.ap[1]])
            ps_kq = kq_ps.tile([128, 512], F32, tag="kq", name="kq")
            for q in range(4):
                gi = sg * 4 + q
                blk = slice(gi * 128, gi * 128 + 128)
                nc.tensor.matmul(ps_kq[:, q * 128:(q + 1) * 128],
                                 kt[:, blk], qt[:, blk],
                                 start=True, stop=True)
            enum_r = sm_pool.tile([128, 512], BF16, tag="enum_r")
            nc.scalar.activation(enum_r, ps_kq,
                                 mybir.ActivationFunctionType.Exp,
                                 scale=NORM)
            enum_m = sm_pool.tile([128, 512], BF16, tag="enum_m")
            nc.vector.tensor_mul(enum_m, enum_r, bm_rep)
            sums = small.tile([128, 4], F32, tag="sums")
            nc.vector.reduce_sum(
                sums, enum_m.rearrange("p (q c) -> p q c", c=128), axis=AX.X)
            recip = small.tile([128, 4], F32, tag="recip")
            nc.vector.reciprocal(recip, sums)
            rc_bc = bass.AP(tensor=recip.tensor, offset=recip.offset,
                            ap=[recip.ap[0], recip.ap[1], [0, 128]])
            smkq = sm_pool.tile([128, 512], BF16, tag="smkq")
            nc.vector.tensor_mul(smkq, enum_m, rc_bc)
            return smkq

        def phase_d_att(h, sg, smkq):
            """att matmuls + atT copy for one supergroup."""
            ps_at = at_ps.tile([128, 512], F32, tag="at", name="at")
            for q in range(4):
                gi = sg * 4 + q
                nc.tensor.matmul(ps_at[:, q * 128:(q + 1) * 128],
                                 vnat[h][:, gi * 128:gi * 128 + 128],
                                 smkq[:, q * 128:(q + 1) * 128],
                                 start=True, stop=True)
            nc.scalar.copy(out=atT[h][:, sg * 512:(sg + 1) * 512],
                           in_=ps_at)

        def phase_d(h, etile_at=None):
            """Software-pipelined supergroups: KQ(sg+1) is emitted before
            att(sg) so the PE has independent work while the softmax chain
            of sg runs on scalar/vector. etile_at: {slot: (ci, tl, col0)}
            E tiles emitted when their atT range is complete (att lags one
            sg behind the slot index)."""
            nsg = NSG[h]
            etile_at = etile_at or {}
            pend = []
            for sg in range(nsg):
                smkq = phase_d_kq(h, sg)
                pend.append((sg, smkq))
                if len(pend) > 2:
                    psg, psm = pend.pop(0)
                    phase_d_att(h, psg, psm)
                if sg in etile_at:
                    ci, tl, col0 = etile_at[sg]
                    phase_e_tile(ci, tl, col0)
            for psg, psm in pend:
                phase_d_att(h, psg, psm)

        # emission order: C per head, software-pipelined D with E tiles
        # interleaved once their atT s-ranges complete (att emitted through
        # slot-1 at slot sg, so tile tl of c1 goes at slot 2tl+2).
        for h in range(H):
            phase_c(h)
        phase_d(0, {3: (0, 0, 0), 5: (0, 1, 512), 7: (0, 2, 1024)})
        phase_e_tile(0, 3, 1536)
        phase_d(1, {3: (1, 0, 2048)})
        phase_e_tile(1, 1, 2048 + 512)
        phase_d(2)
        phase_e_tile(2, 0, 3072)          # needs h0..h2 only
        phase_d(3)
        phase_e_tile(3, 0, 3584)


_NC = None


def _get_program():
    global _NC
    if _NC is None:
        _NC = build_program()
    return _NC


def _host_inputs(Wk, Wq, Wv, W_out, b_out):
    bf = ml_dtypes.bfloat16
    Wstack = np.stack([Wq, Wk, Wv], 1)                     # [H, 3, 128, 1024]
    tmp = Wstack.reshape(H, 3, 128, NECHUNK, 128)          # [h, p, c, ec, r]
    wqkv_sb = np.ascontiguousarray(
        tmp.transpose(4, 0, 1, 3, 2)).reshape(128, -1).astype(bf)
    wout_sb = np.ascontiguousarray(
        W_out.reshape(E, H, 128).transpose(2, 1, 0)).reshape(128, H * E
                                                             ).astype(bf)
    # binary masks: bm[h][p=k*L+l_n, c=k'*L+l_m] = 1 iff k==k' and l_m<=l_n
    bm_host = np.zeros((128, H * 128), bf)
    for h in range(H):
        L = LS[h]
        for p in range(128):
            k, ln = p // L, p % L
            c0 = h * 128 + k * L
            bm_host[p, c0:c0 + ln + 1] = 1.0
    return wqkv_sb, wout_sb, bm_host


def _shard_x(xbf, c):
    """Core c inputs: [chunk, ec, 128, CHN*NS] bf16, col = n_local*64 + s."""
    b, half = c // 2, c % 2
    xsb = xbf[b].reshape(NB, SEG, E)[:, half * NS:(half + 1) * NS, :]
    # [n, s, e] -> [chunk, ec, e_in, n_local*s]
    xsb = xsb.reshape(NCH, CHN * NS, NECHUNK, 128)
    return np.ascontiguousarray(xsb.transpose(0, 2, 3, 1))


def _unpack_y(res_yT, b_out):
    """yT [8, 128, 4096] (class-packed cols) -> y_core [4096, 1024] f32."""
    flat = np.asarray(res_yT, np.float32).transpose(2, 0, 1).reshape(4096, E)
    out = np.empty((4096, E), np.float32)
    out[COL_PERM] = flat
    out += np.asarray(b_out, np.float32).reshape(1, E)
    return out


def kernel(x, Wk, Wq, Wv, W_out, b_out):
    bf = ml_dtypes.bfloat16
    xbf = np.asarray(x, np.float32).astype(bf)
    wqkv_sb, wout_sb, bm_host = _host_inputs(
        np.asarray(Wk, np.float32), np.asarray(Wq, np.float32),
        np.asarray(Wv, np.float32), np.asarray(W_out, np.float32),
        np.asarray(b_out, np.float32))
    in_maps = []
    for c in range(8):
        in_maps.append({"xs": _shard_x(xbf, c), "wqkv": wqkv_sb,
                        "wout": wout_sb, "bm": bm_host})
    nc = _get_program()
    res = run_bass_kernel_spmd(nc, in_maps, core_ids=list(range(8)))
    y = np.empty((B, T, E), np.float32)
    for c in range(8):
        b, half = c // 2, c % 2
        y[b, half * ROWS:(half + 1) * ROWS, :] = _unpack_y(
            res.results[c]["yT"], b_out)
    return y
